# revision 2
# baseline (speedup 1.0000x reference)
"""FP8 fake-quant matmul on 8 TRN2 NeuronCores.

Computes reference semantics:
    w_dq = fq(weight, s_w);  x_dq = fq(x.reshape(-1,K), s_x)
    out  = (x_dq @ w_dq).reshape(B, S, N)
where fq(t, s) = clip(t*s, +-448) round-tripped through float8_e4m3fn (OCP),
s = 448 / amax(|t|).

Device strategy (data-parallel over rows M = B*S, 8 shards):
  Launch A: per-core partial amax of x shard and weight shard -> host combines
            to the exact global fp32 amax / scales.
  Launch B: per-core quantize (TRN e4m3 at HALF scale: TRN fp8e4 max-normal is
            240, not 448 -- x*s/2 <= 224 needs no clip and rounds identically
            to OCP at full scale), PE-transpose x tiles, fp8 matmul accumulate
            fp32 in PSUM, dequant-scale, write out.
"""

import sys

for _p in ("/opt/trn_rl_repo", "/root/.axon_site"):
    if _p not in sys.path:
        sys.path.insert(0, _p)

import numpy as np

import concourse.bass as bass  # noqa: F401  (registers engine classes)
import concourse.tile as tile
from concourse import bacc, mybir
from concourse.bass_utils import run_bass_kernel_spmd
from concourse.masks import make_identity

# Problem shapes (hardcoded per spec)
B, S, K, N = 8, 2048, 4096, 4096
NCORES = 8
MS = (B * S) // NCORES  # 2048 rows of x per core
WS = K // NCORES  # 512 rows of weight per core (amax sharding)
P = 128
FP32 = mybir.dt.float32
FP8 = mybir.dt.float8e4
FP8_MAX = np.float32(448.0)

_CACHE = {}


def _build_amax():
    nc = bacc.Bacc(None, target_bir_lowering=False, debug=False)
    xs = nc.declare_dram_parameter("xs", [MS, K], FP32, isOutput=False)
    ws = nc.declare_dram_parameter("ws", [WS, K], FP32, isOutput=False)
    pm = nc.declare_dram_parameter("pm", [P, 2], FP32, isOutput=True)
    nxt = MS // P  # 16
    nwt = WS // P  # 4
    with tile.TileContext(nc) as tc:
        with (
            tc.tile_pool(name="io", bufs=4) as io,
            tc.tile_pool(name="st", bufs=1) as stp,
        ):
            st = stp.tile([P, nxt + nwt], FP32)
            fin = stp.tile([P, 2], FP32)
            xt = xs[:].rearrange("(t p) k -> t p k", p=P)
            wt = ws[:].rearrange("(t p) k -> t p k", p=P)
            for i in range(nxt):
                t = io.tile([P, K], FP32, tag="io")
                nc.sync.dma_start(out=t[:], in_=xt[i])
                nc.vector.reduce_max(
                    st[:, i : i + 1], t[:], axis=mybir.AxisListType.X,
                    apply_absolute_value=True,
                )
            for i in range(nwt):
                t = io.tile([P, K], FP32, tag="io")
                nc.sync.dma_start(out=t[:], in_=wt[i])
                nc.vector.reduce_max(
                    st[:, nxt + i : nxt + i + 1], t[:], axis=mybir.AxisListType.X,
                    apply_absolute_value=True,
                )
            nc.vector.reduce_max(
                fin[:, 0:1], st[:, 0:nxt], axis=mybir.AxisListType.X
            )
            nc.vector.reduce_max(
                fin[:, 1:2], st[:, nxt : nxt + nwt], axis=mybir.AxisListType.X
            )
            nc.sync.dma_start(out=pm[:], in_=fin[:])
    nc.compile()
    return nc


def _build_main():
    nc = bacc.Bacc(None, target_bir_lowering=False, debug=False)
    xs = nc.declare_dram_parameter("xs", [MS, K], FP32, isOutput=False)
    w = nc.declare_dram_parameter("w", [K, N], FP32, isOutput=False)
    sc = nc.declare_dram_parameter("sc", [1, 8], FP32, isOutput=False)
    out = nc.declare_dram_parameter("out", [MS, N], FP32, isOutput=True)
    MT, KT = MS // P, K // P  # 16, 32
    NB = 512  # psum bank width (fp32)
    with tile.TileContext(nc) as tc:
        with (
            tc.tile_pool(name="const", bufs=1) as cst,
            tc.tile_pool(name="wf", bufs=3) as wfp,
            tc.tile_pool(name="wq", bufs=KT) as wqp,
            tc.tile_pool(name="xf", bufs=2) as xfp,
            tc.tile_pool(name="xqt", bufs=2) as xqtp,
            tc.tile_pool(name="ob", bufs=4) as obp,
            tc.tile_pool(name="tps", bufs=2, space="PSUM") as tpsp,
            tc.tile_pool(name="mps", bufs=4, space="PSUM") as mpsp,
        ):
            ident = cst.tile([P, P], FP32)
            make_identity(nc, ident)
            scs = cst.tile([P, 8], FP32)
            nc.sync.dma_start(out=scs[:], in_=sc[:].to_broadcast([P, 8]))
            sxs = scs[:, 0:1]  # s_x / 2
            sws = scs[:, 1:2]  # s_w / 2
            dqs = scs[:, 2:3]  # 4 / (s_x * s_w) with reference rounding

            # ---- weight: load + quantize to fp8 (resident in SBUF) ----
            wt = w[:].rearrange("(t p) n -> t p n", p=P)  # [32, 128, 4096]
            wqs = []
            for k in range(KT):
                wq = wqp.tile([P, N], FP8, tag="wq")
                for h in range(2):
                    wf = wfp.tile([P, N // 2], FP32, tag="wf")
                    nc.sync.dma_start(
                        out=wf[:], in_=wt[k, :, h * (N // 2) : (h + 1) * (N // 2)]
                    )
                    nc.scalar.mul(
                        wq[:, h * (N // 2) : (h + 1) * (N // 2)], wf[:], sws
                    )
                wqs.append(wq)

            # ---- x: load, PE-transpose, quantize; then fp8 matmul ----
            xt = xs[:].rearrange("(t p) k -> t p k", p=P)  # [16, 128, 4096]
            ot = out[:].rearrange("(t p) n -> t p n", p=P)
            for m in range(MT):
                xf = xfp.tile([P, K], FP32, tag="xf")
                nc.sync.dma_start(out=xf[:], in_=xt[m])
                xqt = xqtp.tile([P, KT, P], FP8, tag="xqt")
                for k in range(KT):
                    ps = tpsp.tile([P, P], FP32, tag="tps")
                    nc.tensor.transpose(ps[:], xf[:, k * P : (k + 1) * P], ident[:])
                    nc.vector.tensor_scalar_mul(xqt[:, k, :], ps[:], sxs)
                for half in range(2):
                    pss = [
                        mpsp.tile([P, NB], FP32, tag="mps", name=f"mps_{m}_{half}_{j}")
                        for j in range(4)
                    ]
                    for j in range(4):
                        n = half * 4 + j
                        for k in range(KT):
                            nc.tensor.matmul(
                                pss[j][:],
                                xqt[:, k, :],
                                wqs[k][:, n * NB : (n + 1) * NB],
                                start=(k == 0),
                                stop=(k == KT - 1),
                            )
                    for j in range(4):
                        n = half * 4 + j
                        ob = obp.tile([P, NB], FP32, tag="ob")
                        nc.vector.tensor_scalar_mul(ob[:], pss[j][:], dqs)
                        nc.sync.dma_start(
                            out=ot[m, :, n * NB : (n + 1) * NB], in_=ob[:]
                        )
    nc.compile()
    return nc


def _get(name, builder):
    if name not in _CACHE:
        _CACHE[name] = builder()
    return _CACHE[name]


def kernel(x: np.ndarray, weight: np.ndarray) -> np.ndarray:
    x = np.ascontiguousarray(np.asarray(x, dtype=np.float32))
    weight = np.ascontiguousarray(np.asarray(weight, dtype=np.float32))
    assert x.shape == (B, S, K) and weight.shape == (K, N)
    x2d = x.reshape(B * S, K)

    core_ids = list(range(NCORES))
    x_shards = [x2d[c * MS : (c + 1) * MS] for c in core_ids]
    w_shards = [weight[c * WS : (c + 1) * WS] for c in core_ids]

    # ---- Launch A: partial amax ----
    nc_a = _get("amax", _build_amax)
    res_a = run_bass_kernel_spmd(
        nc_a,
        [{"xs": x_shards[c], "ws": w_shards[c]} for c in core_ids],
        core_ids,
    )
    pms = np.stack([res_a.results[c]["pm"] for c in core_ids])  # [8, 128, 2]
    amax_x = np.float32(pms[:, :, 0].max())
    amax_w = np.float32(pms[:, :, 1].max())

    # Exact reference scale arithmetic (fp32 throughout)
    s_x = FP8_MAX / np.maximum(amax_x, np.float32(1e-12))
    s_w = FP8_MAX / np.maximum(amax_w, np.float32(1e-12))
    r_x = np.float32(1.0) / s_x
    r_w = np.float32(1.0) / s_w
    dq = np.float32(4.0) * r_x * r_w
    scales = np.zeros((1, 8), np.float32)
    scales[0, 0] = s_x * np.float32(0.5)
    scales[0, 1] = s_w * np.float32(0.5)
    scales[0, 2] = dq

    # ---- Launch B: quantize + matmul ----
    nc_b = _get("main", _build_main)
    res_b = run_bass_kernel_spmd(
        nc_b,
        [{"xs": x_shards[c], "w": weight, "sc": scales} for c in core_ids],
        core_ids,
    )
    out = np.concatenate([res_b.results[c]["out"] for c in core_ids], axis=0)
    return out.reshape(B, S, N)


# revision 3
# speedup vs baseline: 1.5483x; 1.5483x over previous
"""FP8 fake-quant matmul on 8 TRN2 NeuronCores.

Computes reference semantics:
    w_dq = fq(weight, s_w);  x_dq = fq(x.reshape(-1,K), s_x)
    out  = (x_dq @ w_dq).reshape(B, S, N)
where fq(t, s) = clip(t*s, +-448) round-tripped through float8_e4m3fn (OCP),
s = 448 / amax(|t|).

Device strategy (data-parallel over rows M = B*S, 8 shards):
  Launch A: per-core partial amax of x shard and weight shard -> host combines
            to the exact global fp32 amax / scales.
  Launch B: per-core quantize (TRN e4m3 at HALF scale: TRN fp8e4 max-normal is
            240, not 448 -- x*s/2 <= 224 needs no clip and rounds identically
            to OCP at full scale), PE-transpose x tiles, fp8 matmul accumulate
            fp32 in PSUM, dequant-scale, write out.
"""

import sys

for _p in ("/opt/trn_rl_repo", "/root/.axon_site"):
    if _p not in sys.path:
        sys.path.insert(0, _p)

import numpy as np

import concourse.bass as bass  # noqa: F401  (registers engine classes)
import concourse.tile as tile
from concourse import bacc, mybir
from concourse.bass_utils import run_bass_kernel_spmd
from concourse.masks import make_identity

# Problem shapes (hardcoded per spec)
B, S, K, N = 8, 2048, 4096, 4096
NCORES = 8
MS = (B * S) // NCORES  # 2048 rows of x per core
WS = K // NCORES  # 512 rows of weight per core (amax sharding)
P = 128
FP32 = mybir.dt.float32
FP8 = mybir.dt.float8e4
FP8_MAX = np.float32(448.0)

_CACHE = {}


def _build_amax():
    nc = bacc.Bacc(None, target_bir_lowering=False, debug=False)
    xs = nc.declare_dram_parameter("xs", [MS, K], FP32, isOutput=False)
    ws = nc.declare_dram_parameter("ws", [WS, K], FP32, isOutput=False)
    pm = nc.declare_dram_parameter("pm", [P, 2], FP32, isOutput=True)
    nxt = MS // P  # 16
    nwt = WS // P  # 4
    with tile.TileContext(nc) as tc:
        with (
            tc.tile_pool(name="io", bufs=4) as io,
            tc.tile_pool(name="st", bufs=1) as stp,
        ):
            st = stp.tile([P, nxt + nwt], FP32)
            fin = stp.tile([P, 2], FP32)
            xt = xs[:].rearrange("(t p) k -> t p k", p=P)
            wt = ws[:].rearrange("(t p) k -> t p k", p=P)
            for i in range(nxt):
                t = io.tile([P, K], FP32, tag="io")
                nc.sync.dma_start(out=t[:], in_=xt[i])
                nc.vector.reduce_max(
                    st[:, i : i + 1], t[:], axis=mybir.AxisListType.X,
                    apply_absolute_value=True,
                )
            for i in range(nwt):
                t = io.tile([P, K], FP32, tag="io")
                nc.sync.dma_start(out=t[:], in_=wt[i])
                nc.vector.reduce_max(
                    st[:, nxt + i : nxt + i + 1], t[:], axis=mybir.AxisListType.X,
                    apply_absolute_value=True,
                )
            nc.vector.reduce_max(
                fin[:, 0:1], st[:, 0:nxt], axis=mybir.AxisListType.X
            )
            nc.vector.reduce_max(
                fin[:, 1:2], st[:, nxt : nxt + nwt], axis=mybir.AxisListType.X
            )
            nc.sync.dma_start(out=pm[:], in_=fin[:])
    nc.compile()
    return nc


def _build_main():
    nc = bacc.Bacc(None, target_bir_lowering=False, debug=False)
    xs = nc.declare_dram_parameter("xs", [MS, K], FP32, isOutput=False)
    w = nc.declare_dram_parameter("w", [K, N], FP32, isOutput=False)
    sc = nc.declare_dram_parameter("sc", [1, 8], FP32, isOutput=False)
    out = nc.declare_dram_parameter("out", [MS, N], FP32, isOutput=True)
    MT, KT = MS // P, K // P  # 16, 32
    NB = 512  # psum bank width (fp32)
    with tile.TileContext(nc) as tc:
        with (
            tc.tile_pool(name="const", bufs=1) as cst,
            tc.tile_pool(name="wf", bufs=3) as wfp,
            tc.tile_pool(name="wq", bufs=KT) as wqp,
            tc.tile_pool(name="xf", bufs=2) as xfp,
            tc.tile_pool(name="xqt", bufs=2) as xqtp,
            tc.tile_pool(name="ob", bufs=4) as obp,
            tc.tile_pool(name="tps", bufs=2, space="PSUM") as tpsp,
            tc.tile_pool(name="mps", bufs=4, space="PSUM") as mpsp,
        ):
            ident = cst.tile([P, P], FP32)
            make_identity(nc, ident)
            scs = cst.tile([P, 8], FP32)
            nc.sync.dma_start(out=scs[:], in_=sc[:].to_broadcast([P, 8]))
            sxs = scs[:, 0:1]  # s_x / 2
            sws = scs[:, 1:2]  # s_w / 2
            dqs = scs[:, 2:3]  # 4 / (s_x * s_w) with reference rounding

            # ---- weight: load + quantize to fp8 (resident in SBUF) ----
            # DoubleRow layout: chunk c covers k rows [c*256, c*256+256);
            # plane i holds rows c*256 + 128*i + p.  Loaded in column-half
            # order (h) so the first n-half's matmuls can start after only
            # half the weight bytes have arrived.
            NH = N // 2  # 2048
            CT = KT // 2  # 16 chunks of 256 contraction rows
            wt = w[:].rearrange("(t p) n -> t p n", p=P)  # [32, 128, 4096]
            wqs = [[None] * 2 for _ in range(CT)]
            for h in range(2):
                for c in range(CT):
                    wq = wqp.tile([P, 2, NH], FP8, tag="wq", name=f"wq_{c}_{h}")
                    for i in range(2):
                        wf = wfp.tile([P, NH], FP32, tag="wf", name=f"wf_{c}_{h}_{i}")
                        nc.sync.dma_start(
                            out=wf[:], in_=wt[2 * c + i, :, h * NH : (h + 1) * NH]
                        )
                        nc.scalar.mul(wq[:, i, :], wf[:], sws)
                    wqs[c][h] = wq

            # ---- x: load, PE-transpose, quantize; then fp8 matmul ----
            xt = xs[:].rearrange("(t p) k -> t p k", p=P)  # [16, 128, 4096]
            ot = out[:].rearrange("(t p) n -> t p n", p=P)
            DR = mybir.MatmulPerfMode.DoubleRow
            for m in range(MT):
                xf = xfp.tile([P, K], FP32, tag="xf")
                nc.sync.dma_start(out=xf[:], in_=xt[m])
                # slot (c, i) of xqt = transpose of x k-subtile 2c+i, so the
                # (p, i) pairing matches the wq layout above.
                xqt = xqtp.tile([P, CT, 2, P], FP8, tag="xqt")
                for k in range(KT):
                    ps = tpsp.tile([P, P], FP32, tag="tps")
                    nc.tensor.transpose(ps[:], xf[:, k * P : (k + 1) * P], ident[:])
                    nc.vector.tensor_scalar_mul(xqt[:, k // 2, k % 2, :], ps[:], sxs)
                for half in range(2):
                    pss = [
                        mpsp.tile([P, NB], FP32, tag="mps", name=f"mps_{m}_{half}_{j}")
                        for j in range(4)
                    ]
                    for j in range(4):
                        jn = half * (NH // NB) + j  # global n-tile index
                        for c in range(CT):
                            nc.tensor.matmul(
                                pss[j][:],
                                xqt[:, c, :, :],
                                wqs[c][half][:, :, j * NB : (j + 1) * NB],
                                start=(c == 0),
                                stop=(c == CT - 1),
                                perf_mode=DR,
                            )
                    for j in range(4):
                        n = half * 4 + j
                        ob = obp.tile([P, NB], FP32, tag="ob")
                        nc.vector.tensor_scalar_mul(ob[:], pss[j][:], dqs)
                        nc.sync.dma_start(
                            out=ot[m, :, n * NB : (n + 1) * NB], in_=ob[:]
                        )
    nc.compile()
    return nc


def _get(name, builder):
    if name not in _CACHE:
        _CACHE[name] = builder()
    return _CACHE[name]


def kernel(x: np.ndarray, weight: np.ndarray) -> np.ndarray:
    x = np.ascontiguousarray(np.asarray(x, dtype=np.float32))
    weight = np.ascontiguousarray(np.asarray(weight, dtype=np.float32))
    assert x.shape == (B, S, K) and weight.shape == (K, N)
    x2d = x.reshape(B * S, K)

    core_ids = list(range(NCORES))
    x_shards = [x2d[c * MS : (c + 1) * MS] for c in core_ids]
    w_shards = [weight[c * WS : (c + 1) * WS] for c in core_ids]

    # ---- Launch A: partial amax ----
    nc_a = _get("amax", _build_amax)
    res_a = run_bass_kernel_spmd(
        nc_a,
        [{"xs": x_shards[c], "ws": w_shards[c]} for c in core_ids],
        core_ids,
    )
    pms = np.stack([res_a.results[c]["pm"] for c in core_ids])  # [8, 128, 2]
    amax_x = np.float32(pms[:, :, 0].max())
    amax_w = np.float32(pms[:, :, 1].max())

    # Exact reference scale arithmetic (fp32 throughout)
    s_x = FP8_MAX / np.maximum(amax_x, np.float32(1e-12))
    s_w = FP8_MAX / np.maximum(amax_w, np.float32(1e-12))
    r_x = np.float32(1.0) / s_x
    r_w = np.float32(1.0) / s_w
    dq = np.float32(4.0) * r_x * r_w
    scales = np.zeros((1, 8), np.float32)
    scales[0, 0] = s_x * np.float32(0.5)
    scales[0, 1] = s_w * np.float32(0.5)
    scales[0, 2] = dq

    # ---- Launch B: quantize + matmul ----
    nc_b = _get("main", _build_main)
    res_b = run_bass_kernel_spmd(
        nc_b,
        [{"xs": x_shards[c], "w": weight, "sc": scales} for c in core_ids],
        core_ids,
    )
    out = np.concatenate([res_b.results[c]["out"] for c in core_ids], axis=0)
    return out.reshape(B, S, N)
